# revision 4
# baseline (speedup 1.0000x reference)
"""Blockwise 8x8 2D DCT (forward/inverse) on 8 TRN2 NeuronCores.

Reference op: x [B,C,H,W] -> per 8x8 block X: D @ X @ D^T (forward) or
D^T @ X @ D (inverse), with D the 8x8 orthonormal DCT-II matrix.

Strategy (pure data-parallel, batch-sharded across 8 cores):
  Per core the shard is viewed as [rows, W] with rows = (B/8)*C*H.
  For each 128x128 SBUF chunk C the TensorEngine computes
      P1 = C.T @ G        (matmul with C as the stationary operand)
      P2 = P1.T @ G       (matmul with P1 as the stationary operand)
  where G = kron(I_16, Ds.T) is block-diagonal (Ds = D or D.T).  The first
  matmul applies the row (H) transform and transposes the chunk; the second
  applies the column (W) transform and transposes it back.  No explicit
  transposes, 2 matmuls per chunk, all arithmetic in fp32 with fp32 PSUM
  accumulation.

Must be built as bacc.Bacc + nc.compile(): the compile pass legalizes
multi-wait instructions into InstEventSemaphore carriers; raw bass.Bass
programs with >1 sync wait on a Matmult fail walrus codegen.
"""

import numpy as np
from contextlib import ExitStack

P = 128
N_CORES = 8
BLOCK = 8


def _build_nc(rows: int, width: int):
    import concourse.bacc as bacc
    import concourse.mybir as mybir
    import concourse.tile as tile

    nc = bacc.Bacc("TRN2", target_bir_lowering=False, debug=False)
    x = nc.dram_tensor("x", [rows, width], mybir.dt.float32, kind="ExternalInput").ap()
    g = nc.dram_tensor("g", [P, P], mybir.dt.float32, kind="ExternalInput").ap()
    out = nc.dram_tensor(
        "out", [rows, width], mybir.dt.float32, kind="ExternalOutput"
    ).ap()

    n_tiles = rows // P
    n_ch = width // P

    with ExitStack() as ctx:
        tc = ctx.enter_context(tile.TileContext(nc))
        const = ctx.enter_context(tc.tile_pool(name="const", bufs=1))
        xp = ctx.enter_context(tc.tile_pool(name="xp", bufs=4))
        op = ctx.enter_context(tc.tile_pool(name="op", bufs=4))
        s1p = ctx.enter_context(tc.tile_pool(name="s1p", bufs=8))
        p1p = ctx.enter_context(tc.tile_pool(name="p1p", bufs=4, space="PSUM"))
        p2p = ctx.enter_context(tc.tile_pool(name="p2p", bufs=4, space="PSUM"))

        g_t = const.tile([P, P], mybir.dt.float32)
        nc.sync.dma_start(out=g_t[:], in_=g)

        for t in range(n_tiles):
            x_t = xp.tile([P, width], mybir.dt.float32)
            nc.sync.dma_start(out=x_t[:], in_=x[t * P : (t + 1) * P, :])
            o_t = op.tile([P, width], mybir.dt.float32)
            for j in range(n_ch):
                p1 = p1p.tile([P, P], mybir.dt.float32)
                nc.tensor.matmul(
                    p1[:],
                    lhsT=x_t[:, j * P : (j + 1) * P],
                    rhs=g_t[:],
                    start=True,
                    stop=True,
                )
                s1 = s1p.tile([P, P], mybir.dt.float32)
                nc.scalar.copy(s1[:], p1[:])
                p2 = p2p.tile([P, P], mybir.dt.float32)
                nc.tensor.matmul(
                    p2[:], lhsT=s1[:], rhs=g_t[:], start=True, stop=True
                )
                nc.vector.tensor_copy(o_t[:, j * P : (j + 1) * P], p2[:])
            nc.sync.dma_start(out=out[t * P : (t + 1) * P, :], in_=o_t[:])
    nc.compile()
    return nc


def _make_g(dct_mat: np.ndarray, inverse: int) -> np.ndarray:
    D = np.asarray(dct_mat, dtype=np.float32)
    Ds = D if inverse == 0 else D.T
    return np.kron(
        np.eye(P // Ds.shape[0], dtype=np.float32),
        np.ascontiguousarray(Ds.T, dtype=np.float32),
    )


def _run(x, dct_mat, inverse=0, trace=False):
    from concourse.bass_utils import run_bass_kernel_spmd

    x = np.ascontiguousarray(np.asarray(x, dtype=np.float32))
    inv = int(np.asarray(inverse))
    G = _make_g(dct_mat, inv)

    B, C, H, W = x.shape
    per = B // N_CORES
    rows = per * C * H
    shards = x.reshape(N_CORES, rows, W)

    nc = _build_nc(rows, W)
    in_maps = [{"x": shards[i], "g": G} for i in range(N_CORES)]
    res = run_bass_kernel_spmd(
        nc, in_maps, core_ids=list(range(N_CORES)), trace=trace
    )
    y = np.stack([res.results[i]["out"] for i in range(N_CORES)], axis=0)
    return y.reshape(B, C, H, W), res


def kernel(x, dct_mat, inverse=0, **_unused):
    y, _ = _run(x, dct_mat, inverse=inverse, trace=False)
    return y


# revision 8
# speedup vs baseline: 4.6899x; 4.6899x over previous
"""Blockwise 8x8 2D DCT (forward/inverse) on 8 TRN2 NeuronCores.

Reference op: x [B,C,H,W] -> per 8x8 block X: D @ X @ D^T (forward) or
D^T @ X @ D (inverse), with D the 8x8 orthonormal DCT-II matrix.

Strategy (pure data-parallel, batch-sharded across 8 cores):
  Per core the shard is viewed as [rows, W] with rows = (B/8)*C*H.
  For each 128x128 SBUF chunk C the TensorEngine computes
      P1 = C.T @ G        (matmul with C as the stationary operand)
      P2 = P1.T @ G       (matmul with P1 as the stationary operand)
  where G = kron(I_16, Ds.T) is block-diagonal (Ds = D or D.T).  The first
  matmul applies the row (H) transform and transposes the chunk; the second
  applies the column (W) transform and transposes it back.  No explicit
  transposes, 2 matmuls per chunk, all arithmetic in fp32 with fp32 PSUM
  accumulation.

Must be built as bacc.Bacc + nc.compile(): the compile pass legalizes
multi-wait instructions into InstEventSemaphore carriers; raw bass.Bass
programs with >1 sync wait on a Matmult fail walrus codegen.
"""

import numpy as np
from contextlib import ExitStack

P = 128
N_CORES = 8
BLOCK = 8


def _build_nc(rows: int, width: int, repeat: int = 1, col_tile: bool = False):
    # `repeat` re-runs the whole loop inside one NEFF (same output written
    # each time) — used by test.py to measure pure silicon time as a slope
    # between repeat=1 and repeat=R without per-dispatch overhead.
    import concourse.bacc as bacc
    import concourse.mybir as mybir
    import concourse.tile as tile

    nc = bacc.Bacc("TRN2", target_bir_lowering=False, debug=False)
    x = nc.dram_tensor("x", [rows, width], mybir.dt.float32, kind="ExternalInput").ap()
    g = nc.dram_tensor("g", [P, P], mybir.dt.float32, kind="ExternalInput").ap()
    out = nc.dram_tensor(
        "out", [rows, width], mybir.dt.float32, kind="ExternalOutput"
    ).ap()

    n_tiles = rows // P
    n_ch = width // P

    with ExitStack() as ctx:
        tc = ctx.enter_context(tile.TileContext(nc))
        const = ctx.enter_context(tc.tile_pool(name="const", bufs=1))
        xp = ctx.enter_context(tc.tile_pool(name="xp", bufs=4))
        op = ctx.enter_context(tc.tile_pool(name="op", bufs=4))
        s1p = ctx.enter_context(tc.tile_pool(name="s1p", bufs=8))
        p1p = ctx.enter_context(tc.tile_pool(name="p1p", bufs=4, space="PSUM"))
        p2p = ctx.enter_context(tc.tile_pool(name="p2p", bufs=4, space="PSUM"))

        g_t = const.tile([P, P], mybir.dt.float32)
        nc.sync.dma_start(out=g_t[:], in_=g)

        for t in [t for _ in range(repeat) for t in range(n_tiles)]:
            x_t = xp.tile([P, width], mybir.dt.float32)
            nc.sync.dma_start(out=x_t[:], in_=x[t * P : (t + 1) * P, :])
            o_t = op.tile([P, width], mybir.dt.float32)

            def mm(dst, src):
                # dst(PSUM) = src(SBUF).T @ g_t
                if not col_tile:
                    nc.tensor.matmul(
                        dst[:], lhsT=src, rhs=g_t[:], start=True, stop=True
                    )
                else:
                    # 4 concurrent M=32 col-group matmuls: 32-column
                    # LDWEIGHTS (27ns vs 107ns) and per-subarray overlap.
                    for ct in range(4):
                        nc.tensor.matmul(
                            dst[32 * ct : 32 * (ct + 1), :],
                            lhsT=src[:, 32 * ct : 32 * (ct + 1)],
                            rhs=g_t[:],
                            tile_position=(0, 32 * ct),
                            start=True,
                            stop=True,
                        )

            for j in range(n_ch):
                p1 = p1p.tile([P, P], mybir.dt.float32)
                mm(p1, x_t[:, j * P : (j + 1) * P])
                s1 = s1p.tile([P, P], mybir.dt.float32)
                nc.scalar.copy(s1[:], p1[:])
                p2 = p2p.tile([P, P], mybir.dt.float32)
                mm(p2, s1[:])
                nc.vector.tensor_copy(o_t[:, j * P : (j + 1) * P], p2[:])
            nc.sync.dma_start(out=out[t * P : (t + 1) * P, :], in_=o_t[:])
    nc.compile()
    return nc


def _make_g(dct_mat: np.ndarray, inverse: int) -> np.ndarray:
    D = np.asarray(dct_mat, dtype=np.float32)
    Ds = D if inverse == 0 else D.T
    return np.kron(
        np.eye(P // Ds.shape[0], dtype=np.float32),
        np.ascontiguousarray(Ds.T, dtype=np.float32),
    )


def _run(x, dct_mat, inverse=0, trace=False):
    from concourse.bass_utils import run_bass_kernel_spmd

    x = np.ascontiguousarray(np.asarray(x, dtype=np.float32))
    inv = int(np.asarray(inverse))
    G = _make_g(dct_mat, inv)

    B, C, H, W = x.shape
    per = B // N_CORES
    rows = per * C * H
    shards = x.reshape(N_CORES, rows, W)

    nc = _build_nc(rows, W)
    in_maps = [{"x": shards[i], "g": G} for i in range(N_CORES)]
    res = run_bass_kernel_spmd(
        nc, in_maps, core_ids=list(range(N_CORES)), trace=trace
    )
    y = np.stack([res.results[i]["out"] for i in range(N_CORES)], axis=0)
    return y.reshape(B, C, H, W), res


def kernel(x, dct_mat, inverse=0, **_unused):
    y, _ = _run(x, dct_mat, inverse=inverse, trace=False)
    return y
